# revision 13
# baseline (speedup 1.0000x reference)
"""MultiHeadAttention (head-shared scores) on 8 Trainium2 NeuronCores.

kernel(**inputs) takes the FULL inputs
  x [4, 2048, 1024], W_attn [1024, 3072], b_attn [3072],
  W_proj [1024, 1024], b_proj [1024]
and returns the FULL output [4, 2048, 1024] (float32).

Sharding: data-parallel over (batch, sequence-half) -> 8 shards.
Core c handles batch c//2, sequence-half c%2. Every core receives the
full x of its batch, ROTATED so that its own s-half sits at rows 0:1024
(attention output for row s is invariant under any joint permutation of
the k/v rows, so all 8 cores run one identical SPMD program with
s_half = 0; the rotated row order is used consistently for K^T, the
softmax t-range and the w@x contraction, so no un-rotation is needed).
Weights are replicated. b_proj is added on the host.

Per-core program (matmuls in float32r = fp32 data, ~FP22 multiply,
full PE rate; transposes ride a bf16 identity as the moving operand
so they run at 1 cycle/row; everything else fp32). V = x W_v is never
materialized: attn = w (x W_v) = (w x) W_v by associativity. Softmax
is computed WITHOUT max-subtraction (logits = scores/8 with |logit|
<~ 30, exp fits fp32 comfortably) and UNNORMALIZED: the 1/rowsum is
folded into the final output tiles (everything between is linear).
  P1  XT = x^T via PE transposes                 [128, 8, 512] x 2 blocks
  P2  KT local = W_k^T x_s^T; spill to DRAM; pairwise AllReduce(add);
      reload the pair-sum; partner half = sum - own (DVE/Pool).
  P3  QT = W_q^T x_s^T (fills the exchange window)
  P4  scores^T tiles [t_p, s] = KT-slice^T-GEMM vs QT -> exp (Act,
      PSUM->WT directly, no weight transposes); Pool accumulates the
      running column-sum; afterwards 8 small PE transposes + DVE
      reduce give rowsum -> recips [s_p, 8].
  P4b yT = (w~ x)^T via x-row-tiles stationary (w~ unnormalized)
  P5a attnT = W_v^T-GEMM(yT)  (+ rank-1 b_v * sumexp when b_attn != 0)
  P5b out = attnT^T-GEMM(W_proj) scaled by recips -> DMA out
      (b_proj added on host)
"""

import sys
from contextlib import ExitStack

import numpy as np

try:
    import concourse.bass as bass  # noqa: F401
except ImportError:  # pragma: no cover
    sys.path.insert(0, "/opt/trn_rl_repo")

import concourse.bass as bass
import concourse.mybir as mybir
import concourse.tile as tile
from concourse import bacc
from concourse.bass_utils import run_bass_kernel_spmd
from concourse.masks import make_identity

FP32 = mybir.dt.float32
FP32R = mybir.dt.float32r
BF16 = mybir.dt.bfloat16

# timing-model escape hatch: TimelineSim cannot model collectives; setting
# this builds the same program minus the AllReduce instruction (numerically
# wrong, timing-equivalent apart from the collective's own latency).
_SKIP_COLLECTIVE = False

B = 4
P = 128
T = 2048          # full sequence (t range)
S = 1024          # per-core s-half
E = 1024
KE = E // P       # 8 e-tiles
NT = T // P       # 16 t-tiles
TBN = 4           # t-blocks
TBW = T // TBN    # 512 columns per t-block
SM = S // P       # 8 s-tiles
NCH = 512         # matmul moving free-dim chunk
SCALE = 0.125     # 1/sqrt(d_head) = 1/8
N_CORES = 8


def _build_core_program(tc, outs, ins, has_battn: bool):
    """Emit the per-core program (s_half = 0). ins/outs are DRAM APs."""
    nc = tc.nc
    x = ins["x"]            # [2048, 1024] (rows 0:1024 are this core's s rows)
    W_attn = ins["W_attn"]  # [1024, 3072]
    W_proj = ins["W_proj"]  # [1024, 1024]
    out = outs["out"]       # [1024, 1024]

    es_const = ExitStack()
    es_x = ExitStack()
    es_big = ExitStack()
    es_wq = ExitStack()
    es_qt = ExitStack()
    es_wk = ExitStack()
    es_sum = ExitStack()
    es_stat = ExitStack()
    es_wt = ExitStack()
    es_wv = ExitStack()
    es_yt = ExitStack()
    es_at = ExitStack()
    es_wp = ExitStack()
    es_p5 = ExitStack()

    # ---- constant / psum pools (whole kernel) ----
    constp = es_const.enter_context(tc.tile_pool(name="constp", bufs=1, side="left"))
    psA = es_const.enter_context(tc.tile_pool(name="psA", bufs=6, space="PSUM"))
    psT = es_const.enter_context(tc.tile_pool(name="psT", bufs=2, space="PSUM"))

    # fp32r identity: transposes are charged by the MOVING operand's dtype,
    # and the moving operand of a PE transpose is the identity -> 1.5 cyc/row
    # instead of fp32's 2.0 (the BIR verifier requires both matmul inputs to
    # be the same type when either is fp32/fp32r, so bf16 is not an option).
    identf = constp.tile([P, P], FP32)
    make_identity(nc, identf[:])
    identr = constp.tile([P, P], FP32R, tag="identr")
    nc.vector.tensor_copy(identr[:], identf[:])
    ident = identr[:]
    recips = constp.tile([P, SM], FP32, tag="recips")

    if has_battn:
        b_attn = ins["b_attn"]  # [3072]
        # b_attn in free-dim layout on partition 0: [1, 3072]
        b_free = constp.tile([1, 3 * E], FP32R, tag="b_free")
        nc.sync.dma_start(b_free[:], b_attn.rearrange("(a j) -> a j", a=1).bitcast(FP32R))
        ones_row = constp.tile([1, NCH], FP32R, tag="ones_row")
        nc.vector.memset(ones_row[:], 1.0)
        ones_col = constp.tile([P, 1], FP32R, tag="ones_col")
        nc.vector.memset(ones_col[:], 1.0)
        srow = constp.tile([1, S], FP32R, tag="srow")

    # ================= P1: build XT (x^T) via PE transposes =================
    # wk prefetch pool opened below xp on the right stack; its DMAs are
    # emitted after the first x tiles so x loads win the queue race.
    # (K^T-local runs BEFORE Q^T so Q^T fills the exchange window.)
    wqp = es_wq.enter_context(tc.tile_pool(name="wqp", bufs=1, side="right"))
    wq = wqp.tile([P, KE, E], FP32R, tag="wq")
    wkp = es_wk.enter_context(tc.tile_pool(name="wkp", bufs=1, side="right"))
    wk = wkp.tile([P, KE, E], FP32R, tag="wk")
    xp = es_x.enter_context(tc.tile_pool(name="xp", bufs=3, side="right"))
    bigp = es_big.enter_context(tc.tile_pool(name="bigp", bufs=4, side="left"))
    # DRAM bounce buffers for the pairwise K^T exchange
    dramp = es_const.enter_context(tc.tile_pool(name="dramp", bufs=1, space="DRAM"))
    ktl_b = dramp.tile([TBN // 2, P, KE, TBW], FP32R, tag="ktl_b")
    ktsum_b = dramp.tile([TBN // 2, P, KE, TBW], FP32R, tag="ktsum_b")

    xt_blocks = []
    for tb in range(TBN // 2):   # own s-half only
        xt_blocks.append(bigp.tile([P, KE, TBW], FP32R, tag="big", name=f"xt{tb}"))
    for it in range(NT // 2):    # 8 x row-tiles (own half)
        xtile = xp.tile([P, E], FP32R, tag="xtile")
        for xh in range(2):
            nc.sync.dma_start(
                xtile[:, xh * (E // 2) : (xh + 1) * (E // 2)],
                x[it * P : (it + 1) * P, xh * (E // 2) : (xh + 1) * (E // 2)]
                .bitcast(FP32R),
            )
        if it == 3:
            nc.sync.dma_start(
                wk[:, 0 : KE // 2, :],
                W_attn[: E // 2, E : 2 * E].rearrange("(k p) j -> p k j", p=P).bitcast(FP32R),
            )
        if it == 7:
            nc.sync.dma_start(
                wk[:, KE // 2 :, :],
                W_attn[E // 2 :, E : 2 * E].rearrange("(k p) j -> p k j", p=P).bitcast(FP32R),
            )
        tb, toff = it // (TBW // P), (it % (TBW // P)) * P
        for ke in range(KE):
            pt = psT.tile([P, P], FP32R, tag="pst")
            nc.tensor.transpose(pt[:], xtile[:, ke * P : (ke + 1) * P], ident)
            dst = xt_blocks[tb][:, ke, toff : toff + P]
            if ke % 2 == 0:
                nc.vector.tensor_copy(dst, pt[:])
            else:
                nc.scalar.copy(dst, pt[:])
    es_x.close()
    # wq loads during the KTl GEMM (DMA engine is otherwise idle there),
    # so QT is ready to fill the exchange window.
    nc.sync.dma_start(
        wq[:], W_attn[:, 0:E].rearrange("(k p) j -> p k j", p=P).bitcast(FP32R)
    )

    # ==== P2: local KT (own half), pairwise AllReduce, reload pair-sum ====
    # k-split: the k=0..3 half starts as soon as the first wk half lands
    # (~12us) instead of waiting for all of wk (~23us); the k=4..7 half
    # accumulates in PSUM and a DVE/Pool add merges both into ktb.
    es_kh = ExitStack()
    khp = es_kh.enter_context(tc.tile_pool(name="khp", bufs=2, side="right"))
    kth_blocks = []
    for tb in range(TBN // 2):
        xtb = xt_blocks[tb]
        kth = khp.tile([P, KE, TBW], FP32, tag="kth", name=f"kth{tb}")
        kth_blocks.append(kth)
        for m in range(KE):      # e_k tile, contraction half 1
            ps = psA.tile([P, TBW], FP32, tag="psA")
            first = True
            if has_battn:
                nc.tensor.matmul(   # out[i, j] += b_k[m*128+i] * 1
                    ps[:], (b_free[:, E + m * P : E + (m + 1) * P]),
                    (ones_row[:]), start=True, stop=False,
                )
                first = False
            for k in range(KE // 2):
                nc.tensor.matmul(
                    ps[:],
                    (wk[:, k, m * P : (m + 1) * P]),
                    (xtb[:, k, :]),
                    start=first,
                    stop=(k == KE // 2 - 1),
                )
                first = False
            if m % 2 == 0:
                nc.vector.tensor_copy(kth[:, m, :], ps[:])
            else:
                nc.scalar.copy(kth[:, m, :], ps[:])
    ktl_blocks = []
    for tb in range(TBN // 2):
        xtb = xt_blocks[tb]
        ktb = bigp.tile([P, KE, TBW], FP32R, tag="big", name=f"kt{tb}")
        ktl_blocks.append(ktb)
        for m in range(KE):      # e_k tile, contraction half 2 + merge
            ps = psA.tile([P, TBW], FP32, tag="psA")
            for k in range(KE // 2, KE):
                nc.tensor.matmul(
                    ps[:],
                    (wk[:, k, m * P : (m + 1) * P]),
                    (xtb[:, k, :]),
                    start=(k == KE // 2),
                    stop=(k == KE - 1),
                )
            nc.vector.tensor_add(ktb[:, m, :], ps[:], kth_blocks[tb][:, m, :])
            nc.sync.dma_start(ktl_b[tb, :, m, :], ktb[:, m, :])
    es_kh.close()
    es_wk.close()
    if not _SKIP_COLLECTIVE:
        nc.gpsimd.collective_compute(
            "AllReduce",
            mybir.AluOpType.add,
            replica_groups=[[2 * g, 2 * g + 1] for g in range(N_CORES // 2)],
            ins=[ktl_b.opt()],
            outs=[ktsum_b.opt()],
        )
    # reload the pair-sum; partner half = sum - own (in place, off the PE
    # critical path: DVE takes one block, Pool the other)
    sump = es_sum.enter_context(tc.tile_pool(name="sump", bufs=2, side="left"))
    sum_blocks = []
    for i in range(TBN // 2):
        kg = sump.tile([P, KE, TBW], FP32R, tag="sumb", name=f"sum{i}")
        sum_blocks.append(kg)
        for h in range(2):
            sl = slice(h * KE // 2, (h + 1) * KE // 2)
            nc.sync.dma_start(kg[:, sl, :], ktsum_b[i, :, sl, :])
    for i in range(TBN // 2):
        for h in range(2):
            sl = slice(h * KE // 2, (h + 1) * KE // 2)
            eng = nc.vector if (i + h) % 2 == 0 else nc.gpsimd
            eng.tensor_sub(
                sum_blocks[i][:, sl, :],
                sum_blocks[i][:, sl, :],
                ktl_blocks[i][:, sl, :],
            )

    # ================= P3: QT = W_q^T @ x_s^T (fills the exchange window) ===
    qtp = es_qt.enter_context(tc.tile_pool(name="qtp", bufs=1, side="left"))
    qt = qtp.tile([P, KE, S], FP32R, tag="qt")
    # s rows (= t rows 0:1024) live in XT t-blocks 0 and 1
    for m in range(KE):            # output e_q tile (psum partitions)
        for n in range(S // NCH):  # s chunk -> t-block n
            ps = psA.tile([P, NCH], FP32, tag="psA")
            first = True
            if has_battn:
                nc.tensor.matmul(   # out[i, j] += b_q[m*128+i] * 1
                    ps[:], (b_free[:, m * P : (m + 1) * P]),
                    (ones_row[:]), start=True, stop=False,
                )
                first = False
            for k in range(KE):
                nc.tensor.matmul(
                    ps[:],
                    (wq[:, k, m * P : (m + 1) * P]),
                    (xt_blocks[n][:, k, :]),
                    start=first,
                    stop=(k == KE - 1),
                )
                first = False
            nc.scalar.copy(qt[:, m, n * NCH : (n + 1) * NCH], ps[:])
    es_wq.close()

    # ====== P4: scores^T per t-tile -> exp into WT; running column-sum ======
    wtp = es_wt.enter_context(tc.tile_pool(name="wtp", bufs=1, side="right"))
    statp = es_stat.enter_context(tc.tile_pool(name="statp", bufs=1, side="right"))
    wt = wtp.tile([P, NT, S], FP32R, tag="wt")
    acc = statp.tile([P, S], FP32R, tag="acc")

    for tt in range(NT):
        blk = ktl_blocks[tt // (TBW // P)] if tt < NT // 2 else \
            sum_blocks[tt // (TBW // P) - TBN // 2]
        to = (tt % (TBW // P)) * P
        for c in range(S // NCH):
            ps = psA.tile([P, NCH], FP32, tag="psA")
            for k in range(KE):
                nc.tensor.matmul(
                    ps[:],
                    (blk[:, k, to : to + P]),
                    (qt[:, k, c * NCH : (c + 1) * NCH]),
                    start=(k == 0),
                    stop=(k == KE - 1),
                )
            # exp((scores)*SCALE), unnormalized, straight into WT
            nc.scalar.activation(
                wt[:, tt, c * NCH : (c + 1) * NCH],
                ps[:],
                mybir.ActivationFunctionType.Exp,
                scale=SCALE,
            )
        # running column-sum on the (otherwise idle) Pool engine
        if tt == 0:
            nc.gpsimd.tensor_copy(acc[:], wt[:, 0, :])
        else:
            nc.gpsimd.tensor_add(acc[:], acc[:], wt[:, tt, :])
    es_qt.close()

    # rowsum over t = column-sum of acc over partitions: 8 small transposes
    sumst = statp.tile([P, SM], FP32, tag="sumst")
    for b in range(SM):
        pt = psT.tile([P, P], FP32R, tag="pst")
        nc.tensor.transpose(pt[:], acc[:, b * P : (b + 1) * P], ident)
        nc.vector.reduce_sum(
            sumst[:, b : b + 1], pt[:].bitcast(FP32), axis=mybir.AxisListType.X
        )
    nc.vector.reciprocal(recips[:], sumst[:])
    if has_battn:
        # sumexp as a [1, S] row for the rank-1 b_v correction in P5a
        pssr = psA.tile([1, S], FP32, tag="psA")
        for b in range(SM):
            nc.tensor.matmul(
                pssr[:, b * P : (b + 1) * P],
                (ones_col[:]),
                (acc[:, b * P : (b + 1) * P]),
                start=True,
                stop=True,
            )
        nc.scalar.copy(srow[:], pssr[:])
    es_stat.close()
    es_sum.close()

    # ====== P4b: yT = (w~ x)^T via x-row-tiles as stationary ======
    # x natural chunks live in freed bigp slots (XT slots die after QT,
    # own-KT slots die as the last own scores tiles consume them), so
    # their DMAs start during P4 instead of after it. x is already in
    # this core's rotated row order == wt's t order.
    xn = []
    for g in range(4):   # chunked load of x in natural layout, rotated order
        xng = bigp.tile([P, NT // 4, E], FP32R, tag="big", name=f"xn{g}")
        xn.append(xng)
        for h in range(2):
            nc.sync.dma_start(
                xng[:, h * 2 : (h + 1) * 2, :],
                x[(g * 4 + h * 2) * P : (g * 4 + h * 2 + 2) * P, :]
                .rearrange("(kt p) e -> p kt e", p=P)
                .bitcast(FP32R),
            )
    wvp = es_wv.enter_context(tc.tile_pool(name="wvp", bufs=1, side="left"))
    wv = wvp.tile([P, KE, E], FP32R, tag="wv")
    nc.sync.dma_start(
        wv[:],
        W_attn[:, 2 * E : 3 * E].rearrange("(k p) j -> p k j", p=P).bitcast(FP32R),
    )
    ytp = es_yt.enter_context(tc.tile_pool(name="ytp", bufs=1, side="left"))
    yt = ytp.tile([P, KE, S], FP32R, tag="yt")
    for m in range(KE):          # e tile of y^T partitions
        for n in range(S // NCH):
            ps = psA.tile([P, NCH], FP32, tag="psA")
            for kt in range(NT):
                nc.tensor.matmul(
                    ps[:],
                    (xn[kt // 4][:, kt % 4, m * P : (m + 1) * P]),
                    (wt[:, kt, n * NCH : (n + 1) * NCH]),
                    start=(kt == 0),
                    stop=(kt == NT - 1),
                )
            nc.scalar.copy(yt[:, m, n * NCH : (n + 1) * NCH], ps[:])
    es_wt.close()

    # ====== P5a: attnT = W_v^T y^T (+ rank-1 b_v * sumexp) ======
    atp = es_at.enter_context(tc.tile_pool(name="atp", bufs=1, side="right"))
    wpp = es_wp.enter_context(tc.tile_pool(name="wpp", bufs=1, side="right"))
    wp = wpp.tile([P, KE, E], FP32R, tag="wp")
    nc.sync.dma_start(wp[:], W_proj.rearrange("(k p) j -> p k j", p=P).bitcast(FP32R))
    at = atp.tile([P, KE, S], FP32R, tag="at")
    for m in range(KE):          # e_v tile of attn^T partitions
        for n in range(S // NCH):
            ps = psA.tile([P, NCH], FP32, tag="psA")
            first = True
            if has_battn:
                nc.tensor.matmul(   # out[i, j] += b_v[m*128+i] * sumexp[j]
                    ps[:], (b_free[:, 2 * E + m * P : 2 * E + (m + 1) * P]),
                    (srow[:, n * NCH : (n + 1) * NCH]), start=True, stop=False,
                )
                first = False
            for k in range(KE):
                nc.tensor.matmul(
                    ps[:],
                    (wv[:, k, m * P : (m + 1) * P]),
                    (yt[:, k, n * NCH : (n + 1) * NCH]),
                    start=first,
                    stop=(k == KE - 1),
                )
                first = False
            nc.scalar.copy(at[:, m, n * NCH : (n + 1) * NCH], ps[:])
    es_yt.close()
    es_wv.close()
    es_big.close()

    # ====== P5b: out = (attn~ @ W_proj) * recip (b_proj added on host) ======
    outbp = es_p5.enter_context(tc.tile_pool(name="outbp", bufs=2, side="right"))
    for ms in range(SM):
        ob = outbp.tile([P, E], FP32, tag="ob")
        for n in range(E // NCH):
            ps = psA.tile([P, NCH], FP32, tag="psA")
            for k in range(KE):
                nc.tensor.matmul(
                    ps[:],
                    (at[:, k, ms * P : (ms + 1) * P]),
                    (wp[:, k, n * NCH : (n + 1) * NCH]),
                    start=(k == 0),
                    stop=(k == KE - 1),
                )
            if n % 2 == 0:
                nc.vector.tensor_scalar_mul(
                    ob[:, n * NCH : (n + 1) * NCH], ps[:], recips[:, ms : ms + 1]
                )
            else:
                nc.scalar.activation(
                    ob[:, n * NCH : (n + 1) * NCH],
                    ps[:],
                    mybir.ActivationFunctionType.Copy,
                    scale=recips[:, ms : ms + 1],
                )
            nc.sync.dma_start(
                out[ms * P : (ms + 1) * P, n * NCH : (n + 1) * NCH],
                ob[:, n * NCH : (n + 1) * NCH],
            )
    es_p5.close()
    es_wp.close()
    es_at.close()
    es_const.close()


_MODULE_CACHE = {}


def _build_module(has_battn: bool):
    if has_battn in _MODULE_CACHE:
        return _MODULE_CACHE[has_battn]
    nc = bacc.Bacc(
        "TRN2", target_bir_lowering=False, debug=False, num_devices=N_CORES
    )
    ins = {
        "x": nc.dram_tensor("x", (T, E), FP32, kind="ExternalInput").ap(),
        "W_attn": nc.dram_tensor(
            "W_attn", (E, 3 * E), FP32, kind="ExternalInput"
        ).ap(),
        "W_proj": nc.dram_tensor(
            "W_proj", (E, E), FP32, kind="ExternalInput"
        ).ap(),
    }
    if has_battn:
        ins["b_attn"] = nc.dram_tensor(
            "b_attn", (3 * E,), FP32, kind="ExternalInput"
        ).ap()
    outs = {"out": nc.dram_tensor("out", (S, E), FP32, kind="ExternalOutput").ap()}
    with tile.TileContext(nc) as tc:
        _build_core_program(tc, outs, ins, has_battn)
    nc.compile()
    _MODULE_CACHE[has_battn] = nc
    return nc


def _make_in_maps(x, W_attn, b_attn, W_proj, has_battn):
    in_maps = []
    for c in range(N_CORES):
        b, j = c // 2, c % 2
        xb = x[b]
        if j == 0:
            x_core = np.ascontiguousarray(xb)
        else:
            # rotate so this core's s-half sits at rows 0:1024
            x_core = np.ascontiguousarray(np.roll(xb, -S, axis=0))
        m = {"x": x_core, "W_attn": W_attn, "W_proj": W_proj}
        if has_battn:
            m["b_attn"] = b_attn
        in_maps.append(m)
    return in_maps


def run_on_cores(x, W_attn, b_attn, W_proj, b_proj, trace=False, **trace_kwargs):
    """Build, compile, run on cores 0-7; returns (out_full, BassKernelResults)."""
    x = np.asarray(x, np.float32)
    W_attn = np.asarray(W_attn, np.float32)
    b_attn = np.asarray(b_attn, np.float32)
    W_proj = np.asarray(W_proj, np.float32)
    b_proj = np.asarray(b_proj, np.float32)

    has_battn = bool(np.any(b_attn))
    nc = _build_module(has_battn)

    in_maps = _make_in_maps(x, W_attn, b_attn, W_proj, has_battn)

    # the axon terminal occasionally drops a fresh process's first execute
    # (worker hung up / NRT unrecoverable); retry a couple of times.
    last_exc = None
    for attempt in range(3):
        try:
            res = run_bass_kernel_spmd(
                nc, in_maps, core_ids=list(range(N_CORES)), trace=trace,
                **trace_kwargs
            )
            break
        except Exception as e:  # noqa: BLE001
            last_exc = e
            import time as _time
            _time.sleep(2.0)
    else:
        raise last_exc

    out = np.empty((B, T, E), np.float32)
    for c in range(N_CORES):
        b, j = c // 2, c % 2
        out[b, j * S : (j + 1) * S, :] = res.results[c]["out"]
    out += b_proj[None, None, :]
    return out, res


def kernel(**inputs):
    out, _ = run_on_cores(
        inputs["x"],
        inputs["W_attn"],
        inputs["b_attn"],
        inputs["W_proj"],
        inputs["b_proj"],
        trace=False,
    )
    return out


# revision 14
# speedup vs baseline: 1.0198x; 1.0198x over previous
"""MultiHeadAttention (head-shared scores) on 8 Trainium2 NeuronCores.

kernel(**inputs) takes the FULL inputs
  x [4, 2048, 1024], W_attn [1024, 3072], b_attn [3072],
  W_proj [1024, 1024], b_proj [1024]
and returns the FULL output [4, 2048, 1024] (float32).

Sharding: data-parallel over (batch, sequence-half) -> 8 shards.
Core c handles batch c//2, sequence-half c%2. Every core receives the
full x of its batch, ROTATED so that its own s-half sits at rows 0:1024
(attention output for row s is invariant under any joint permutation of
the k/v rows, so all 8 cores run one identical SPMD program with
s_half = 0; the rotated row order is used consistently for K^T, the
softmax t-range and the w@x contraction, so no un-rotation is needed).
Weights are replicated. b_proj is added on the host.

Per-core program (matmuls in float32r = fp32 data, ~FP22 multiply,
full PE rate; transposes ride a bf16 identity as the moving operand
so they run at 1 cycle/row; everything else fp32). V = x W_v is never
materialized: attn = w (x W_v) = (w x) W_v by associativity. Softmax
is computed WITHOUT max-subtraction (logits = scores/8 with |logit|
<~ 30, exp fits fp32 comfortably) and UNNORMALIZED: the 1/rowsum is
folded into the final output tiles (everything between is linear).
  P1  XT = x^T via PE transposes                 [128, 8, 512] x 2 blocks
  P2  KT local = W_k^T x_s^T; spill to DRAM; pairwise AllReduce(add);
      reload the pair-sum; partner half = sum - own (DVE/Pool).
  P3  QT = W_q^T x_s^T (fills the exchange window)
  P4  scores^T tiles [t_p, s] = KT-slice^T-GEMM vs QT -> exp (Act,
      PSUM->WT directly, no weight transposes); Pool accumulates the
      running column-sum; afterwards 8 small PE transposes + DVE
      reduce give rowsum -> recips [s_p, 8].
  P4b yT = (w~ x)^T via x-row-tiles stationary (w~ unnormalized)
  P5a attnT = W_v^T-GEMM(yT)  (+ rank-1 b_v * sumexp when b_attn != 0)
  P5b out = attnT^T-GEMM(W_proj) scaled by recips -> DMA out
      (b_proj added on host)
"""

import sys
from contextlib import ExitStack

import numpy as np

try:
    import concourse.bass as bass  # noqa: F401
except ImportError:  # pragma: no cover
    sys.path.insert(0, "/opt/trn_rl_repo")

import concourse.bass as bass
import concourse.mybir as mybir
import concourse.tile as tile
from concourse import bacc
from concourse.bass_utils import run_bass_kernel_spmd
from concourse.masks import make_identity

FP32 = mybir.dt.float32
FP32R = mybir.dt.float32r
BF16 = mybir.dt.bfloat16

# timing-model escape hatch: TimelineSim cannot model collectives; setting
# this builds the same program minus the AllReduce instruction (numerically
# wrong, timing-equivalent apart from the collective's own latency).
_SKIP_COLLECTIVE = False

B = 4
P = 128
T = 2048          # full sequence (t range)
S = 1024          # per-core s-half
E = 1024
KE = E // P       # 8 e-tiles
NT = T // P       # 16 t-tiles
TBN = 4           # t-blocks
TBW = T // TBN    # 512 columns per t-block
SM = S // P       # 8 s-tiles
NCH = 512         # matmul moving free-dim chunk
SCALE = 0.125     # 1/sqrt(d_head) = 1/8
N_CORES = 8


def _build_core_program(tc, outs, ins, has_battn: bool):
    """Emit the per-core program (s_half = 0). ins/outs are DRAM APs."""
    nc = tc.nc
    x = ins["x"]            # [2048, 1024] (rows 0:1024 are this core's s rows)
    W_attn = ins["W_attn"]  # [1024, 3072]
    W_proj = ins["W_proj"]  # [1024, 1024]
    out = outs["out"]       # [1024, 1024]

    es_const = ExitStack()
    es_x = ExitStack()
    es_big = ExitStack()
    es_wq = ExitStack()
    es_qt = ExitStack()
    es_wk = ExitStack()
    es_sum = ExitStack()
    es_stat = ExitStack()
    es_wt = ExitStack()
    es_wv = ExitStack()
    es_yt = ExitStack()
    es_at = ExitStack()
    es_wp = ExitStack()
    es_p5 = ExitStack()

    # ---- constant / psum pools (whole kernel) ----
    constp = es_const.enter_context(tc.tile_pool(name="constp", bufs=1, side="left"))
    psA = es_const.enter_context(tc.tile_pool(name="psA", bufs=6, space="PSUM"))
    psT = es_const.enter_context(tc.tile_pool(name="psT", bufs=2, space="PSUM"))

    # fp32r identity: transposes are charged by the MOVING operand's dtype,
    # and the moving operand of a PE transpose is the identity -> 1.5 cyc/row
    # instead of fp32's 2.0 (the BIR verifier requires both matmul inputs to
    # be the same type when either is fp32/fp32r, so bf16 is not an option).
    identf = constp.tile([P, P], FP32)
    make_identity(nc, identf[:])
    identr = constp.tile([P, P], FP32R, tag="identr")
    nc.vector.tensor_copy(identr[:], identf[:])
    ident = identr[:]
    recips = constp.tile([P, SM], FP32, tag="recips")

    if has_battn:
        b_attn = ins["b_attn"]  # [3072]
        # b_attn in free-dim layout on partition 0: [1, 3072]
        b_free = constp.tile([1, 3 * E], FP32R, tag="b_free")
        nc.sync.dma_start(b_free[:], b_attn.rearrange("(a j) -> a j", a=1).bitcast(FP32R))
        ones_row = constp.tile([1, NCH], FP32R, tag="ones_row")
        nc.vector.memset(ones_row[:], 1.0)
        ones_col = constp.tile([P, 1], FP32R, tag="ones_col")
        nc.vector.memset(ones_col[:], 1.0)
        srow = constp.tile([1, S], FP32R, tag="srow")

    # ================= P1: build XT (x^T) via PE transposes =================
    # wk prefetch pool opened below xp on the right stack; its DMAs are
    # emitted after the first x tiles so x loads win the queue race.
    # (K^T-local runs BEFORE Q^T so Q^T fills the exchange window.)
    wqp = es_wq.enter_context(tc.tile_pool(name="wqp", bufs=1, side="right"))
    wq = wqp.tile([P, KE, E], FP32R, tag="wq")
    wkp = es_wk.enter_context(tc.tile_pool(name="wkp", bufs=1, side="right"))
    wk = wkp.tile([P, KE, E], FP32R, tag="wk")
    xp = es_x.enter_context(tc.tile_pool(name="xp", bufs=3, side="right"))
    bigp = es_big.enter_context(tc.tile_pool(name="bigp", bufs=4, side="left"))
    # DRAM bounce buffers for the pairwise K^T exchange
    dramp = es_const.enter_context(tc.tile_pool(name="dramp", bufs=1, space="DRAM"))
    ktl_b = dramp.tile([TBN // 2, P, KE, TBW], FP32R, tag="ktl_b")
    ktsum_b = dramp.tile([TBN // 2, P, KE, TBW], FP32R, tag="ktsum_b")

    xt_blocks = []
    for tb in range(TBN // 2):   # own s-half only
        xt_blocks.append(bigp.tile([P, KE, TBW], FP32R, tag="big", name=f"xt{tb}"))
    for it in range(NT // 2):    # 8 x row-tiles (own half)
        xtile = xp.tile([P, E], FP32R, tag="xtile")
        for xh in range(2):
            nc.sync.dma_start(
                xtile[:, xh * (E // 2) : (xh + 1) * (E // 2)],
                x[it * P : (it + 1) * P, xh * (E // 2) : (xh + 1) * (E // 2)]
                .bitcast(FP32R),
            )
        if it == 3:
            nc.sync.dma_start(
                wk[:, 0 : KE // 2, :],
                W_attn[: E // 2, E : 2 * E].rearrange("(k p) j -> p k j", p=P).bitcast(FP32R),
            )
        if it == 5:
            nc.sync.dma_start(
                wk[:, KE // 2 :, :],
                W_attn[E // 2 :, E : 2 * E].rearrange("(k p) j -> p k j", p=P).bitcast(FP32R),
            )
        tb, toff = it // (TBW // P), (it % (TBW // P)) * P
        for ke in range(KE):
            pt = psT.tile([P, P], FP32R, tag="pst")
            nc.tensor.transpose(pt[:], xtile[:, ke * P : (ke + 1) * P], ident)
            dst = xt_blocks[tb][:, ke, toff : toff + P]
            if ke % 2 == 0:
                nc.vector.tensor_copy(dst, pt[:])
            else:
                nc.scalar.copy(dst, pt[:])
    es_x.close()
    # wq loads during the KTl GEMM (DMA engine is otherwise idle there),
    # so QT is ready to fill the exchange window.
    nc.sync.dma_start(
        wq[:], W_attn[:, 0:E].rearrange("(k p) j -> p k j", p=P).bitcast(FP32R)
    )

    # ==== P2: local KT (own half), pairwise AllReduce, reload pair-sum ====
    ktl_blocks = []
    for tb in range(TBN // 2):
        xtb = xt_blocks[tb]
        ktb = bigp.tile([P, KE, TBW], FP32R, tag="big", name=f"kt{tb}")
        ktl_blocks.append(ktb)
        for m in range(KE):      # e_k tile
            ps = psA.tile([P, TBW], FP32, tag="psA")
            first = True
            if has_battn:
                nc.tensor.matmul(   # out[i, j] += b_k[m*128+i] * 1
                    ps[:], (b_free[:, E + m * P : E + (m + 1) * P]),
                    (ones_row[:]), start=True, stop=False,
                )
                first = False
            for k in range(KE):
                nc.tensor.matmul(
                    ps[:],
                    (wk[:, k, m * P : (m + 1) * P]),
                    (xtb[:, k, :]),
                    start=first,
                    stop=(k == KE - 1),
                )
                first = False
            if m % 2 == 0:
                nc.vector.tensor_copy(ktb[:, m, :], ps[:])
            else:
                nc.scalar.copy(ktb[:, m, :], ps[:])
            nc.sync.dma_start(ktl_b[tb, :, m, :], ktb[:, m, :])
    es_wk.close()
    if not _SKIP_COLLECTIVE:
        nc.gpsimd.collective_compute(
            "AllReduce",
            mybir.AluOpType.add,
            replica_groups=[[2 * g, 2 * g + 1] for g in range(N_CORES // 2)],
            ins=[ktl_b.opt()],
            outs=[ktsum_b.opt()],
        )
    # reload the pair-sum; partner half = sum - own (in place, off the PE
    # critical path: DVE takes one block, Pool the other)
    sump = es_sum.enter_context(tc.tile_pool(name="sump", bufs=2, side="left"))
    sum_blocks = []
    for i in range(TBN // 2):
        kg = sump.tile([P, KE, TBW], FP32R, tag="sumb", name=f"sum{i}")
        sum_blocks.append(kg)
        for h in range(2):
            sl = slice(h * KE // 2, (h + 1) * KE // 2)
            nc.sync.dma_start(kg[:, sl, :], ktsum_b[i, :, sl, :])
    for i in range(TBN // 2):
        for h in range(2):
            sl = slice(h * KE // 2, (h + 1) * KE // 2)
            eng = nc.vector if (i + h) % 2 == 0 else nc.gpsimd
            eng.tensor_sub(
                sum_blocks[i][:, sl, :],
                sum_blocks[i][:, sl, :],
                ktl_blocks[i][:, sl, :],
            )

    # ================= P3: QT = W_q^T @ x_s^T (fills the exchange window) ===
    qtp = es_qt.enter_context(tc.tile_pool(name="qtp", bufs=1, side="left"))
    qt = qtp.tile([P, KE, S], FP32R, tag="qt")
    # s rows (= t rows 0:1024) live in XT t-blocks 0 and 1
    for m in range(KE):            # output e_q tile (psum partitions)
        for n in range(S // NCH):  # s chunk -> t-block n
            ps = psA.tile([P, NCH], FP32, tag="psA")
            first = True
            if has_battn:
                nc.tensor.matmul(   # out[i, j] += b_q[m*128+i] * 1
                    ps[:], (b_free[:, m * P : (m + 1) * P]),
                    (ones_row[:]), start=True, stop=False,
                )
                first = False
            for k in range(KE):
                nc.tensor.matmul(
                    ps[:],
                    (wq[:, k, m * P : (m + 1) * P]),
                    (xt_blocks[n][:, k, :]),
                    start=first,
                    stop=(k == KE - 1),
                )
                first = False
            nc.scalar.copy(qt[:, m, n * NCH : (n + 1) * NCH], ps[:])
    es_wq.close()

    # ====== P4: scores^T per t-tile -> exp into WT; running column-sum ======
    wtp = es_wt.enter_context(tc.tile_pool(name="wtp", bufs=1, side="right"))
    statp = es_stat.enter_context(tc.tile_pool(name="statp", bufs=1, side="right"))
    wt = wtp.tile([P, NT, S], FP32R, tag="wt")
    acc = statp.tile([P, S], FP32R, tag="acc")

    for tt in range(NT):
        blk = ktl_blocks[tt // (TBW // P)] if tt < NT // 2 else \
            sum_blocks[tt // (TBW // P) - TBN // 2]
        to = (tt % (TBW // P)) * P
        for c in range(S // NCH):
            ps = psA.tile([P, NCH], FP32, tag="psA")
            for k in range(KE):
                nc.tensor.matmul(
                    ps[:],
                    (blk[:, k, to : to + P]),
                    (qt[:, k, c * NCH : (c + 1) * NCH]),
                    start=(k == 0),
                    stop=(k == KE - 1),
                )
            # exp((scores)*SCALE), unnormalized, straight into WT
            nc.scalar.activation(
                wt[:, tt, c * NCH : (c + 1) * NCH],
                ps[:],
                mybir.ActivationFunctionType.Exp,
                scale=SCALE,
            )
        # running column-sum on the (otherwise idle) Pool engine
        if tt == 0:
            nc.gpsimd.tensor_copy(acc[:], wt[:, 0, :])
        else:
            nc.gpsimd.tensor_add(acc[:], acc[:], wt[:, tt, :])
    es_qt.close()

    # rowsum over t = column-sum of acc over partitions: 8 small transposes
    sumst = statp.tile([P, SM], FP32, tag="sumst")
    for b in range(SM):
        pt = psT.tile([P, P], FP32R, tag="pst")
        nc.tensor.transpose(pt[:], acc[:, b * P : (b + 1) * P], ident)
        nc.vector.reduce_sum(
            sumst[:, b : b + 1], pt[:].bitcast(FP32), axis=mybir.AxisListType.X
        )
    nc.vector.reciprocal(recips[:], sumst[:])
    if has_battn:
        # sumexp as a [1, S] row for the rank-1 b_v correction in P5a
        pssr = psA.tile([1, S], FP32, tag="psA")
        for b in range(SM):
            nc.tensor.matmul(
                pssr[:, b * P : (b + 1) * P],
                (ones_col[:]),
                (acc[:, b * P : (b + 1) * P]),
                start=True,
                stop=True,
            )
        nc.scalar.copy(srow[:], pssr[:])
    es_stat.close()
    es_sum.close()

    # ====== P4b: yT = (w~ x)^T via x-row-tiles as stationary ======
    # x natural chunks live in freed bigp slots (XT slots die after QT,
    # own-KT slots die as the last own scores tiles consume them), so
    # their DMAs start during P4 instead of after it. x is already in
    # this core's rotated row order == wt's t order.
    xn = []
    for g in range(4):   # chunked load of x in natural layout, rotated order
        xng = bigp.tile([P, NT // 4, E], FP32R, tag="big", name=f"xn{g}")
        xn.append(xng)
        for h in range(2):
            nc.sync.dma_start(
                xng[:, h * 2 : (h + 1) * 2, :],
                x[(g * 4 + h * 2) * P : (g * 4 + h * 2 + 2) * P, :]
                .rearrange("(kt p) e -> p kt e", p=P)
                .bitcast(FP32R),
            )
    wvp = es_wv.enter_context(tc.tile_pool(name="wvp", bufs=1, side="left"))
    wv = wvp.tile([P, KE, E], FP32R, tag="wv")
    nc.sync.dma_start(
        wv[:],
        W_attn[:, 2 * E : 3 * E].rearrange("(k p) j -> p k j", p=P).bitcast(FP32R),
    )
    ytp = es_yt.enter_context(tc.tile_pool(name="ytp", bufs=1, side="left"))
    yt = ytp.tile([P, KE, S], FP32R, tag="yt")
    for m in range(KE):          # e tile of y^T partitions
        for n in range(S // NCH):
            ps = psA.tile([P, NCH], FP32, tag="psA")
            for kt in range(NT):
                nc.tensor.matmul(
                    ps[:],
                    (xn[kt // 4][:, kt % 4, m * P : (m + 1) * P]),
                    (wt[:, kt, n * NCH : (n + 1) * NCH]),
                    start=(kt == 0),
                    stop=(kt == NT - 1),
                )
            nc.scalar.copy(yt[:, m, n * NCH : (n + 1) * NCH], ps[:])
    es_wt.close()

    # ====== P5a: attnT = W_v^T y^T (+ rank-1 b_v * sumexp) ======
    atp = es_at.enter_context(tc.tile_pool(name="atp", bufs=1, side="right"))
    wpp = es_wp.enter_context(tc.tile_pool(name="wpp", bufs=1, side="right"))
    wp = wpp.tile([P, KE, E], FP32R, tag="wp")
    nc.sync.dma_start(wp[:], W_proj.rearrange("(k p) j -> p k j", p=P).bitcast(FP32R))
    at = atp.tile([P, KE, S], FP32R, tag="at")
    for m in range(KE):          # e_v tile of attn^T partitions
        for n in range(S // NCH):
            ps = psA.tile([P, NCH], FP32, tag="psA")
            first = True
            if has_battn:
                nc.tensor.matmul(   # out[i, j] += b_v[m*128+i] * sumexp[j]
                    ps[:], (b_free[:, 2 * E + m * P : 2 * E + (m + 1) * P]),
                    (srow[:, n * NCH : (n + 1) * NCH]), start=True, stop=False,
                )
                first = False
            for k in range(KE):
                nc.tensor.matmul(
                    ps[:],
                    (wv[:, k, m * P : (m + 1) * P]),
                    (yt[:, k, n * NCH : (n + 1) * NCH]),
                    start=first,
                    stop=(k == KE - 1),
                )
                first = False
            nc.scalar.copy(at[:, m, n * NCH : (n + 1) * NCH], ps[:])
    es_yt.close()
    es_wv.close()
    es_big.close()

    # ====== P5b: out = (attn~ @ W_proj) * recip (b_proj added on host) ======
    outbp = es_p5.enter_context(tc.tile_pool(name="outbp", bufs=2, side="right"))
    for ms in range(SM):
        ob = outbp.tile([P, E], FP32, tag="ob")
        for n in range(E // NCH):
            ps = psA.tile([P, NCH], FP32, tag="psA")
            for k in range(KE):
                nc.tensor.matmul(
                    ps[:],
                    (at[:, k, ms * P : (ms + 1) * P]),
                    (wp[:, k, n * NCH : (n + 1) * NCH]),
                    start=(k == 0),
                    stop=(k == KE - 1),
                )
            if n % 2 == 0:
                nc.vector.tensor_scalar_mul(
                    ob[:, n * NCH : (n + 1) * NCH], ps[:], recips[:, ms : ms + 1]
                )
            else:
                nc.scalar.activation(
                    ob[:, n * NCH : (n + 1) * NCH],
                    ps[:],
                    mybir.ActivationFunctionType.Copy,
                    scale=recips[:, ms : ms + 1],
                )
            nc.sync.dma_start(
                out[ms * P : (ms + 1) * P, n * NCH : (n + 1) * NCH],
                ob[:, n * NCH : (n + 1) * NCH],
            )
    es_p5.close()
    es_wp.close()
    es_at.close()
    es_const.close()


_MODULE_CACHE = {}


def _build_module(has_battn: bool):
    if has_battn in _MODULE_CACHE:
        return _MODULE_CACHE[has_battn]
    nc = bacc.Bacc(
        "TRN2", target_bir_lowering=False, debug=False, num_devices=N_CORES
    )
    ins = {
        "x": nc.dram_tensor("x", (T, E), FP32, kind="ExternalInput").ap(),
        "W_attn": nc.dram_tensor(
            "W_attn", (E, 3 * E), FP32, kind="ExternalInput"
        ).ap(),
        "W_proj": nc.dram_tensor(
            "W_proj", (E, E), FP32, kind="ExternalInput"
        ).ap(),
    }
    if has_battn:
        ins["b_attn"] = nc.dram_tensor(
            "b_attn", (3 * E,), FP32, kind="ExternalInput"
        ).ap()
    outs = {"out": nc.dram_tensor("out", (S, E), FP32, kind="ExternalOutput").ap()}
    with tile.TileContext(nc) as tc:
        _build_core_program(tc, outs, ins, has_battn)
    nc.compile()
    _MODULE_CACHE[has_battn] = nc
    return nc


def _make_in_maps(x, W_attn, b_attn, W_proj, has_battn):
    in_maps = []
    for c in range(N_CORES):
        b, j = c // 2, c % 2
        xb = x[b]
        if j == 0:
            x_core = np.ascontiguousarray(xb)
        else:
            # rotate so this core's s-half sits at rows 0:1024
            x_core = np.ascontiguousarray(np.roll(xb, -S, axis=0))
        m = {"x": x_core, "W_attn": W_attn, "W_proj": W_proj}
        if has_battn:
            m["b_attn"] = b_attn
        in_maps.append(m)
    return in_maps


def run_on_cores(x, W_attn, b_attn, W_proj, b_proj, trace=False, **trace_kwargs):
    """Build, compile, run on cores 0-7; returns (out_full, BassKernelResults)."""
    x = np.asarray(x, np.float32)
    W_attn = np.asarray(W_attn, np.float32)
    b_attn = np.asarray(b_attn, np.float32)
    W_proj = np.asarray(W_proj, np.float32)
    b_proj = np.asarray(b_proj, np.float32)

    has_battn = bool(np.any(b_attn))
    nc = _build_module(has_battn)

    in_maps = _make_in_maps(x, W_attn, b_attn, W_proj, has_battn)

    # the axon terminal occasionally drops a fresh process's first execute
    # (worker hung up / NRT unrecoverable); retry a couple of times.
    last_exc = None
    for attempt in range(3):
        try:
            res = run_bass_kernel_spmd(
                nc, in_maps, core_ids=list(range(N_CORES)), trace=trace,
                **trace_kwargs
            )
            break
        except Exception as e:  # noqa: BLE001
            last_exc = e
            import time as _time
            _time.sleep(2.0)
    else:
        raise last_exc

    out = np.empty((B, T, E), np.float32)
    for c in range(N_CORES):
        b, j = c // 2, c % 2
        out[b, j * S : (j + 1) * S, :] = res.results[c]["out"]
    out += b_proj[None, None, :]
    return out, res


def kernel(**inputs):
    out, _ = run_on_cores(
        inputs["x"],
        inputs["W_attn"],
        inputs["b_attn"],
        inputs["W_proj"],
        inputs["b_proj"],
        trace=False,
    )
    return out
